# revision 4
# baseline (speedup 1.0000x reference)
"""MoE (top-2 of 8 experts, SwiGLU) Trainium2 kernel.

Strategy (expert-parallel over 8 NeuronCores):
  * Host: router GEMM + top-2 + sigmoid gates in numpy (selection matches the
    jax fp32 reference on these inputs), then gather each expert's tokens into
    a transposed, capacity-padded bf16 buffer xT_e [H, C]. One expert per core.
  * Device (SPMD, per core): fully fused single pass in bf16.
    All weights (Wg, Wu, Wd ~ 12.6 MB bf16) stay SBUF-resident; the h
    intermediate never leaves SBUF (no DRAM spill).  Per 512-token segment:
      A: for each of 16 I-blocks: pg = x@Wg, pu = x@Wu (PSUM, 8 k-matmuls
         each), then h = silu(pg)*pu via Scalar(Silu) + one DVE mul with a
         bf16 output into SBUF.
      B: yT = (h @ Wd) * gate in two half-passes of 4 output blocks
         (4 PSUM banks each; A double-buffers the other 4), gates applied
         during PSUM eviction into a wide staging tile, one store per pass.
    bf16 matters twice: LDWEIGHTS of a 128x128 bf16 stationary (~0.1us with
    FWL) hides fully under the 213ns/512-row matmul stream, whereas fp32r
    stationary loads (~224ns) gate the pipe; and all weight/x DMA halves.
  * DMA: loads are batched with 3D access patterns (one descriptor-generation
    instruction per 256-column weight slice / per token segment) because each
    dma_start costs ~0.7us of sequencer issue time; ~25 loads total.
  * HAM warmup: ~26 dummy matmuls on a zeroed scratch tile run during the
    startup DMA so the PE clock (1.2 GHz cold -> 2.4 GHz after ~3.4us busy)
    is warm when the real matmuls start.
  * Host: out[idx_e] += yT_e[:, :n_e].T  (indices within one expert are
    unique, so fancy-index += is safe).
"""

import os
import numpy as np
import ml_dtypes

T, H, I, E, TOPK = 8192, 1024, 2048, 8, 2
NCORES = 8
PB = 128
KB = H // PB     # 8 contraction blocks over H
IB = I // PB     # 16 blocks over I
HB = H // PB     # 8 output blocks over H
WCOL = 256       # Wg/Wu load-slice column width (2 I-blocks per slice)
NWC = I // WCOL  # 8 column slices
NWARM = 10       # HAM warmup matmuls (bridge the ~8us preamble+DMA window)

_compiled = {}
last_results = None  # BassKernelResults of the most recent run (for test harness)


def _tsegs(C):
    """Split C into 512-wide segments plus a remainder segment."""
    segs = []
    t0 = 0
    while C - t0 >= 512:
        segs.append((t0, 512))
        t0 += 512
    if C - t0:
        segs.append((t0, C - t0))
    return segs


def _build(C):
    import concourse.bacc as bacc
    import concourse.mybir as mybir
    import concourse.tile as tile

    fp32 = mybir.dt.float32
    bf16 = mybir.dt.bfloat16
    AF = mybir.ActivationFunctionType

    nc = bacc.Bacc("TRN2", target_bir_lowering=False, debug=False,
                   num_devices=NCORES)
    xT = nc.dram_tensor("xT", [H, C], bf16, kind="ExternalInput").ap()
    gm = nc.dram_tensor("gm", [PB, C], fp32, kind="ExternalInput").ap()
    Wg = nc.dram_tensor("Wg", [H, I], bf16, kind="ExternalInput").ap()
    Wu = nc.dram_tensor("Wu", [H, I], bf16, kind="ExternalInput").ap()
    Wd = nc.dram_tensor("Wd", [I, H], bf16, kind="ExternalInput").ap()
    yT = nc.dram_tensor("yT", [H, C], fp32, kind="ExternalOutput").ap()

    # Batched-DMA views: partition-major with the k/ib block index as a free
    # axis, so one dma_start moves all 8 (16) row-blocks of a column slice.
    Wg3 = Wg.rearrange("(k p) j -> p k j", k=KB)     # [128, 8, 2048]
    Wu3 = Wu.rearrange("(k p) j -> p k j", k=KB)
    Wd3 = Wd.rearrange("(b p) j -> p b j", b=IB)     # [128, 16, 1024]
    xT3 = xT.rearrange("(k p) t -> p k t", k=KB)     # [128, 8, C]
    yT3 = yT.rearrange("(b p) t -> p b t", b=HB)     # [128, 8, C]

    segs = _tsegs(C)
    NSEG = len(segs)
    HH = HB // 2

    with tile.TileContext(nc) as tc, \
         tc.tile_pool(name="wp", bufs=1) as wp, \
         tc.tile_pool(name="xp", bufs=1) as xp, \
         tc.tile_pool(name="hp", bufs=2) as hp, \
         tc.tile_pool(name="evA", bufs=2) as evA, \
         tc.tile_pool(name="evB", bufs=2) as evB, \
         tc.tile_pool(name="psA", bufs=2, space="PSUM") as psA, \
         tc.tile_pool(name="psB", bufs=1, space="PSUM") as psB:

        # --- HAM warmup: dummy matmuls on a zeroed scratch tile. ---
        sc = wp.tile([PB, 512], bf16, name="scwarm")
        nc.vector.memset(sc[:], 0)
        pw = psA.tile([PB, 512], fp32, tag="pg", name="pgw")
        for _ in range(NWARM):
            nc.tensor.matmul(pw[:], sc[:, 0:PB], sc[:], start=True, stop=True)

        # --- Loads (sync queue, FIFO), startup-critical first. ---
        t0_0, tb_0 = segs[0]
        xt_s = [None] * NSEG
        xt0 = xp.tile([PB, KB * tb_0], bf16, name="xt0")
        nc.sync.dma_start(
            out=xt0[:].rearrange("p (k t) -> p k t", k=KB),
            in_=xT3[:, :, t0_0:t0_0 + tb_0])
        xt_s[0] = xt0
        wg_s = [None] * NWC
        wu_s = [None] * NWC
        for c in range(NWC):
            wgc = wp.tile([PB, KB * WCOL], bf16, name=f"wg{c}")
            nc.sync.dma_start(
                out=wgc[:].rearrange("p (k j) -> p k j", k=KB),
                in_=Wg3[:, :, c * WCOL:(c + 1) * WCOL])
            wg_s[c] = wgc
            wuc = wp.tile([PB, KB * WCOL], bf16, name=f"wu{c}")
            nc.sync.dma_start(
                out=wuc[:].rearrange("p (k j) -> p k j", k=KB),
                in_=Wu3[:, :, c * WCOL:(c + 1) * WCOL])
            wu_s[c] = wuc
        if NSEG > 1:
            t0_1, tb_1 = segs[1]
            xt1 = xp.tile([PB, KB * tb_1], bf16, name="xt1")
            nc.sync.dma_start(
                out=xt1[:].rearrange("p (k t) -> p k t", k=KB),
                in_=xT3[:, :, t0_1:t0_1 + tb_1])
            xt_s[1] = xt1
        wd = wp.tile([PB, IB * H], bf16, name="wd")
        nc.sync.dma_start(
            out=wd[:].rearrange("p (b j) -> p b j", b=IB),
            in_=Wd3[:, :, :])
        gt = wp.tile([PB, C], fp32, name="gt")
        nc.sync.dma_start(out=gt[:], in_=gm[:])
        for si in range(2, NSEG):
            t0s, tbs = segs[si]
            xts = xp.tile([PB, KB * tbs], bf16, name=f"xt{si}")
            nc.sync.dma_start(
                out=xts[:].rearrange("p (k t) -> p k t", k=KB),
                in_=xT3[:, :, t0s:t0s + tbs])
            xt_s[si] = xts

        # --- Compute: per segment, A (h into SBUF) then B (two half passes).
        for si, (t0, tb) in enumerate(segs):
            xts = xt_s[si]
            h_tiles = []
            for ib in range(IB):
                c, j = ib // 2, (ib % 2) * PB
                pg = psA.tile([PB, tb], fp32, tag="pg", name="pg")
                pu = psA.tile([PB, tb], fp32, tag="pu", name="pu")
                for k in range(KB):
                    nc.tensor.matmul(
                        pg[:], wg_s[c][:, k * WCOL + j:k * WCOL + j + PB],
                        xts[:, k * tb:(k + 1) * tb],
                        start=(k == 0), stop=(k == KB - 1))
                for k in range(KB):
                    nc.tensor.matmul(
                        pu[:], wu_s[c][:, k * WCOL + j:k * WCOL + j + PB],
                        xts[:, k * tb:(k + 1) * tb],
                        start=(k == 0), stop=(k == KB - 1))
                sg = evA.tile([PB, tb], fp32, tag="sg", name="sg")
                nc.scalar.activation(sg[:], pg[:], AF.Silu)
                hh = hp.tile([PB, tb], bf16, tag=f"h{ib}", name=f"h{ib}")
                nc.vector.tensor_mul(hh[:], sg[:], pu[:])
                h_tiles.append(hh)
            for half in range(2):
                pys = [psB.tile([PB, tb], fp32, tag=f"py{j}", name=f"py{j}")
                       for j in range(HH)]
                ytp = evB.tile([PB, HH * tb], fp32, tag="yt", name="yt")
                for ib in range(IB):
                    last = ib == IB - 1
                    for j in range(HH):
                        hb = half * HH + j
                        nc.tensor.matmul(
                            pys[j][:],
                            wd[:, ib * H + hb * PB:ib * H + hb * PB + PB],
                            h_tiles[ib][:],
                            start=(ib == 0), stop=last)
                        if last:
                            nc.vector.tensor_mul(
                                ytp[:, j * tb:(j + 1) * tb], pys[j][:],
                                gt[:, t0:t0 + tb])
                eng = (nc.sync if si == NSEG - 1 else nc.gpsimd)
                eng.dma_start(
                    out=yT3[:, half * HH:(half + 1) * HH, t0:t0 + tb],
                    in_=ytp[:].rearrange("p (b t) -> p b t", b=HH))
    nc.compile()
    return nc


def _route(x, Wr, br):
    """Replicate the reference's fp32 router bit-compatibly on host."""
    logits = x @ Wr + br                       # fp32 GEMM
    order = np.argsort(-logits, axis=1, kind="stable")  # ties -> lowest index
    topk_idx = order[:, :TOPK]
    topk_vals = np.take_along_axis(logits, topk_idx, axis=1)
    g = 1.0 / (1.0 + np.exp(-topk_vals.astype(np.float32)))
    g = g / (np.sum(g, axis=-1, keepdims=True) + 1e-10)
    return topk_idx, g.astype(np.float32)


def kernel(x, Wr, br, Wg, Wu, Wd):
    global last_results
    from concourse.bass_utils import run_bass_kernel_spmd

    x = np.asarray(x, dtype=np.float32)
    Wr = np.asarray(Wr, dtype=np.float32)
    br = np.asarray(br, dtype=np.float32)
    Wg = np.asarray(Wg, dtype=np.float32)
    Wu = np.asarray(Wu, dtype=np.float32)
    Wd = np.asarray(Wd, dtype=np.float32)

    topk_idx, g = _route(x, Wr, br)

    # Per-expert token lists
    idx_lists = []
    gate_lists = []
    for e in range(E):
        mask = topk_idx == e                    # [T, K]
        tok = np.nonzero(mask.any(axis=1))[0]
        gsel = np.where(mask[tok, 0], g[tok, 0], g[tok, 1]).astype(np.float32)
        idx_lists.append(tok.astype(np.int64))
        gate_lists.append(gsel)

    counts = [len(ix) for ix in idx_lists]
    C = max(512, -(-max(counts) // 8) * 8)     # pad to a multiple of 8

    if C not in _compiled:
        _compiled[C] = _build(C)
    nc = _compiled[C]

    bf = ml_dtypes.bfloat16
    xTf = np.ascontiguousarray(x.T).astype(bf)   # [H, T] bf16
    in_maps = []
    for e in range(E):
        n = counts[e]
        xTe = np.zeros((H, C), dtype=bf)
        xTe[:, :n] = xTf[:, idx_lists[e]]
        gme = np.zeros((PB, C), dtype=np.float32)
        gme[:, :n] = gate_lists[e][None, :]
        in_maps.append({
            "xT": xTe,
            "gm": gme,
            "Wg": Wg[e].astype(bf),
            "Wu": Wu[e].astype(bf),
            "Wd": Wd[e].astype(bf),
        })

    trace = bool(int(os.environ.get("MOE_TRACE", "0")))
    trace_cores = (list(range(NCORES))
                   if os.environ.get("MOE_TRACE_ALL") else None)
    last_results = run_bass_kernel_spmd(
        nc, in_maps, core_ids=list(range(NCORES)), trace=trace,
        trace_cores=trace_cores)

    out = np.zeros((T, H), dtype=np.float32)
    for e in range(E):
        n = counts[e]
        yTe = last_results.results[e]["yT"]
        out[idx_lists[e]] += yTe[:, :n].T
    return out


# revision 8
# speedup vs baseline: 1.0434x; 1.0434x over previous
"""MoE (top-2 of 8 experts, SwiGLU) Trainium2 kernel — load-balanced variant.

Like kernel.py (fused single-pass bf16, expert-parallel), but caps the
primary per-core token count at CAP=2048 (= T*TOPK/NCORES, perfectly
balanced) instead of padding every core to the largest expert's count.
Overflow tokens (expert count - CAP, ~300 total here) are split into
single-expert chunks of <= OV tokens, one chunk per core, processed as a
"secondary" pass with that chunk's expert weights streamed from DRAM:

  * secondary A interleaves with the last primary segment's A at i-block
    granularity (16 extra matmuls of OV<=64 moving rows per block, using
    streamed wg2/wu2 column slices);
  * secondary B interleaves with the last primary segment's B half-pass 0
    (8 matmuls per i-block into a single packed PSUM bank py2[128, 8*OV]).

PSUM budget (8 banks): psA pg+pu (2) + psA2 packed pg2|pu2 (1) + psB
py0..3 (4) + py2 (1).  Weights for the secondary expert stream through
small rotating pools, so SBUF still fits everything.

Net: per-core token slots drop from max_e(n_e) (~2184) to CAP+OV (~2096).
"""

import os
import numpy as np
import ml_dtypes

T, H, I, E, TOPK = 8192, 1024, 2048, 8, 2
NCORES = 8
PB = 128
KB = H // PB     # 8 contraction blocks over H
IB = I // PB     # 16 blocks over I
HB = H // PB     # 8 output blocks over H
WCOL = 256       # Wg/Wu load-slice column width (2 I-blocks per slice)
NWC = I // WCOL  # 8 column slices
NWARM = 10       # HAM warmup matmuls (bridge the ~8us preamble+DMA window)
CAP = 2048       # primary per-core token capacity (4 x 512 segments)

_compiled = {}
last_results = None  # BassKernelResults of the most recent run (for test harness)


def _build(OV):
    """Build the SPMD program for CAP primary + OV secondary token slots."""
    import concourse.bacc as bacc
    import concourse.mybir as mybir
    import concourse.tile as tile

    fp32 = mybir.dt.float32
    bf16 = mybir.dt.bfloat16
    AF = mybir.ActivationFunctionType

    CTOT = CAP + OV
    nc = bacc.Bacc("TRN2", target_bir_lowering=False, debug=False,
                   num_devices=NCORES)
    xT = nc.dram_tensor("xT", [H, CTOT], bf16, kind="ExternalInput").ap()
    gm = nc.dram_tensor("gm", [PB, CTOT], fp32, kind="ExternalInput").ap()
    Wg = nc.dram_tensor("Wg", [H, I], bf16, kind="ExternalInput").ap()
    Wu = nc.dram_tensor("Wu", [H, I], bf16, kind="ExternalInput").ap()
    Wd = nc.dram_tensor("Wd", [I, H], bf16, kind="ExternalInput").ap()
    Wg2 = nc.dram_tensor("Wg2", [H, I], bf16, kind="ExternalInput").ap()
    Wu2 = nc.dram_tensor("Wu2", [H, I], bf16, kind="ExternalInput").ap()
    Wd2 = nc.dram_tensor("Wd2", [I, H], bf16, kind="ExternalInput").ap()
    gm2 = nc.dram_tensor("gm2", [PB, HB * OV], fp32, kind="ExternalInput").ap()
    yT = nc.dram_tensor("yT", [H, CTOT], fp32, kind="ExternalOutput").ap()

    Wg3 = Wg.rearrange("(k p) j -> p k j", k=KB)     # [128, 8, 2048]
    Wu3 = Wu.rearrange("(k p) j -> p k j", k=KB)
    Wd3 = Wd.rearrange("(b p) j -> p b j", b=IB)     # [128, 16, 1024]
    Wg23 = Wg2.rearrange("(k p) j -> p k j", k=KB)
    Wu23 = Wu2.rearrange("(k p) j -> p k j", k=KB)
    Wd23 = Wd2.rearrange("(b p) j -> p b j", b=IB)
    xT3 = xT.rearrange("(k p) t -> p k t", k=KB)     # [128, 8, CTOT]
    yT3 = yT.rearrange("(b p) t -> p b t", b=HB)     # [128, 8, CTOT]

    NSEG = CAP // 512                                # 4 primary segments
    HH = HB // 2

    with tile.TileContext(nc) as tc, \
         tc.tile_pool(name="wp", bufs=1) as wp, \
         tc.tile_pool(name="xp", bufs=1) as xp, \
         tc.tile_pool(name="hp", bufs=1) as hp, \
         tc.tile_pool(name="hp2", bufs=1) as hp2, \
         tc.tile_pool(name="evA", bufs=2) as evA, \
         tc.tile_pool(name="evA2", bufs=2) as evA2, \
         tc.tile_pool(name="evB", bufs=1) as evB, \
         tc.tile_pool(name="evB2", bufs=1) as evB2, \
         tc.tile_pool(name="w2g", bufs=6) as w2g, \
         tc.tile_pool(name="w2u", bufs=6) as w2u, \
         tc.tile_pool(name="w2d", bufs=4) as w2d, \
         tc.tile_pool(name="psA", bufs=1, space="PSUM") as psA, \
         tc.tile_pool(name="psA2", bufs=1, space="PSUM") as psA2, \
         tc.tile_pool(name="psB", bufs=1, space="PSUM") as psB, \
         tc.tile_pool(name="psB2", bufs=1, space="PSUM") as psB2:

        # --- HAM warmup: dummy matmuls on a zeroed scratch tile. ---
        sc = wp.tile([PB, 512], bf16, name="scwarm")
        nc.gpsimd.memset(sc[:], 0)
        pw = psA.tile([PB, 512], fp32, tag="pg", name="pgw")
        for _ in range(NWARM):
            nc.tensor.matmul(pw[:], sc[:, 0:PB], sc[:], start=True, stop=True)

        # --- Loads (sync queue, FIFO), startup-critical first. ---
        xt_s = [None] * NSEG
        xt0 = xp.tile([PB, KB * 512], bf16, name="xt0")
        nc.sync.dma_start(
            out=xt0[:].rearrange("p (k t) -> p k t", k=KB),
            in_=xT3[:, :, 0:512])
        xt_s[0] = xt0
        wg_s = [None] * NWC
        wu_s = [None] * NWC
        for c in range(NWC):
            wgc = wp.tile([PB, KB * WCOL], bf16, name=f"wg{c}")
            nc.sync.dma_start(
                out=wgc[:].rearrange("p (k j) -> p k j", k=KB),
                in_=Wg3[:, :, c * WCOL:(c + 1) * WCOL])
            wg_s[c] = wgc
            wuc = wp.tile([PB, KB * WCOL], bf16, name=f"wu{c}")
            nc.sync.dma_start(
                out=wuc[:].rearrange("p (k j) -> p k j", k=KB),
                in_=Wu3[:, :, c * WCOL:(c + 1) * WCOL])
            wu_s[c] = wuc
        xt1 = xp.tile([PB, KB * 512], bf16, name="xt1")
        nc.sync.dma_start(
            out=xt1[:].rearrange("p (k t) -> p k t", k=KB),
            in_=xT3[:, :, 512:1024])
        xt_s[1] = xt1
        wd = wp.tile([PB, IB * H], bf16, name="wd")
        nc.sync.dma_start(
            out=wd[:].rearrange("p (b j) -> p b j", b=IB),
            in_=Wd3[:, :, :])
        gt = wp.tile([PB, CTOT], fp32, name="gt")
        nc.sync.dma_start(out=gt[:], in_=gm[:])
        gt2 = wp.tile([PB, HB * OV], fp32, name="gt2")
        nc.sync.dma_start(out=gt2[:], in_=gm2[:])
        for si in range(2, NSEG):
            xts = xp.tile([PB, KB * 512], bf16, name=f"xt{si}")
            nc.sync.dma_start(
                out=xts[:].rearrange("p (k t) -> p k t", k=KB),
                in_=xT3[:, :, si * 512:(si + 1) * 512])
            xt_s[si] = xts
        xsec = xp.tile([PB, KB * OV], bf16, name="xsec")
        nc.sync.dma_start(
            out=xsec[:].rearrange("p (k t) -> p k t", k=KB),
            in_=xT3[:, :, CAP:CTOT])
        # Secondary expert weights: small rotating pools, consumed during
        # the last primary segment.  The slab-rotation WAR dependencies
        # self-pace the queue.
        wg2_t = []
        wu2_t = []
        for ib in range(IB):
            wg2k = w2g.tile([PB, KB * PB], bf16, tag="wg2", name=f"wg2_{ib}")
            nc.sync.dma_start(
                out=wg2k[:].rearrange("p (k j) -> p k j", k=KB),
                in_=Wg23[:, :, ib * PB:(ib + 1) * PB])
            wg2_t.append(wg2k)
            wu2k = w2u.tile([PB, KB * PB], bf16, tag="wu2", name=f"wu2_{ib}")
            nc.sync.dma_start(
                out=wu2k[:].rearrange("p (k j) -> p k j", k=KB),
                in_=Wu23[:, :, ib * PB:(ib + 1) * PB])
            wu2_t.append(wu2k)
        wd2_t = []
        for ib in range(IB):
            wd2k = w2d.tile([PB, H], bf16, tag="wd2", name=f"wd2_{ib}")
            nc.sync.dma_start(out=wd2k[:], in_=Wd23[:, ib, :])
            wd2_t.append(wd2k)

        # --- Compute: per segment, A (h into SBUF) then B (two half passes).
        # The last segment carries the interleaved secondary pass.
        for si in range(NSEG):
            t0, tb = si * 512, 512
            sec = si == NSEG - 1
            xts = xt_s[si]
            h_tiles = []
            h2_tiles = []
            for ib in range(IB):
                c, j = ib // 2, (ib % 2) * PB
                pg = psA.tile([PB, tb], fp32, tag="pg", name="pg")
                pu = psA.tile([PB, tb], fp32, tag="pu", name="pu")
                for k in range(KB):
                    nc.tensor.matmul(
                        pg[:], wg_s[c][:, k * WCOL + j:k * WCOL + j + PB],
                        xts[:, k * tb:(k + 1) * tb],
                        start=(k == 0), stop=(k == KB - 1))
                for k in range(KB):
                    nc.tensor.matmul(
                        pu[:], wu_s[c][:, k * WCOL + j:k * WCOL + j + PB],
                        xts[:, k * tb:(k + 1) * tb],
                        start=(k == 0), stop=(k == KB - 1))
                sg = evA.tile([PB, tb], fp32, tag="sg", name="sg")
                nc.scalar.activation(sg[:], pg[:], AF.Silu)
                hh = hp.tile([PB, tb], bf16, tag=f"h{ib}", name=f"h{ib}")
                nc.vector.tensor_mul(hh[:], sg[:], pu[:])
                h_tiles.append(hh)
                if sec:
                    # secondary A for this i-block (streamed wg2/wu2)
                    # pg2|pu2 share one PSUM bank: only the bank's first
                    # matmul may set start (start clears the whole bank);
                    # unwritten elements overwrite via has_written bits.
                    pq2 = psA2.tile([PB, 2 * OV], fp32, tag="pq2", name="pq2")
                    for k in range(KB):
                        nc.tensor.matmul(
                            pq2[:, 0:OV], wg2_t[ib][:, k * PB:(k + 1) * PB],
                            xsec[:, k * OV:(k + 1) * OV],
                            start=(k == 0), stop=(k == KB - 1),
                            skip_group_check=True)
                    for k in range(KB):
                        nc.tensor.matmul(
                            pq2[:, OV:2 * OV],
                            wu2_t[ib][:, k * PB:(k + 1) * PB],
                            xsec[:, k * OV:(k + 1) * OV],
                            start=False, stop=(k == KB - 1),
                            skip_group_check=True)
                    sg2 = evA2.tile([PB, OV], fp32, tag="sg2", name="sg2")
                    nc.scalar.activation(sg2[:], pq2[:, 0:OV], AF.Silu)
                    hh2 = hp2.tile([PB, OV], bf16, tag=f"g{ib}", name=f"g{ib}")
                    nc.vector.tensor_mul(hh2[:], sg2[:], pq2[:, OV:2 * OV])
                    h2_tiles.append(hh2)
            py2 = None
            for half in range(2):
                final = sec and half == 1
                pys = [psB.tile([PB, tb], fp32, tag=f"py{j}", name=f"py{j}")
                       for j in range(HH)]
                ytp = evB.tile([PB, HH * tb], fp32, tag="yt", name="yt")
                if sec and half == 0:
                    py2 = psB2.tile([PB, HB * OV], fp32, tag="py2", name="py2")
                for ib in range(IB):
                    last = ib == IB - 1
                    for j in range(HH):
                        hb = half * HH + j
                        nc.tensor.matmul(
                            pys[j][:],
                            wd[:, ib * H + hb * PB:ib * H + hb * PB + PB],
                            h_tiles[ib][:],
                            start=(ib == 0), stop=last)
                        if last:
                            nc.vector.tensor_mul(
                                ytp[:, j * tb:(j + 1) * tb], pys[j][:],
                                gt[:, t0:t0 + tb])
                            if final:
                                eng = (nc.sync, nc.scalar,
                                       nc.gpsimd, nc.sync)[j]
                                eng.dma_start(
                                    out=yT3[:, hb, t0:t0 + tb],
                                    in_=ytp[:, j * tb:(j + 1) * tb])
                    if sec and half == 0:
                        # secondary B for this i-block (streamed wd2).
                        # All 8 hb slices share the py2 bank: only the very
                        # first matmul sets start (clears the bank).
                        for hb in range(HB):
                            nc.tensor.matmul(
                                py2[:, hb * OV:(hb + 1) * OV],
                                wd2_t[ib][:, hb * PB:(hb + 1) * PB],
                                h2_tiles[ib][:],
                                start=(ib == 0 and hb == 0), stop=last,
                                skip_group_check=True)
                if not final:
                    nc.gpsimd.dma_start(
                        out=yT3[:, half * HH:(half + 1) * HH, t0:t0 + tb],
                        in_=ytp[:].rearrange("p (b t) -> p b t", b=HH))
                if sec and half == 0:
                    yt2 = evB2.tile([PB, HB * OV], fp32, tag="yt2", name="yt2")
                    nc.vector.tensor_mul(yt2[:], py2[:], gt2[:])
                    nc.sync.dma_start(
                        out=yT3[:, :, CAP:CTOT],
                        in_=yt2[:].rearrange("p (b t) -> p b t", b=HB))
    nc.compile()
    return nc


def _route(x, Wr, br):
    """Replicate the reference's fp32 router bit-compatibly on host."""
    logits = x @ Wr + br                       # fp32 GEMM
    order = np.argsort(-logits, axis=1, kind="stable")  # ties -> lowest index
    topk_idx = order[:, :TOPK]
    topk_vals = np.take_along_axis(logits, topk_idx, axis=1)
    g = 1.0 / (1.0 + np.exp(-topk_vals.astype(np.float32)))
    g = g / (np.sum(g, axis=-1, keepdims=True) + 1e-10)
    return topk_idx, g.astype(np.float32)


def kernel(x, Wr, br, Wg, Wu, Wd):
    global last_results
    from concourse.bass_utils import run_bass_kernel_spmd

    x = np.asarray(x, dtype=np.float32)
    Wr = np.asarray(Wr, dtype=np.float32)
    br = np.asarray(br, dtype=np.float32)
    Wg = np.asarray(Wg, dtype=np.float32)
    Wu = np.asarray(Wu, dtype=np.float32)
    Wd = np.asarray(Wd, dtype=np.float32)

    topk_idx, g = _route(x, Wr, br)

    idx_lists = []
    gate_lists = []
    for e in range(E):
        mask = topk_idx == e                    # [T, K]
        tok = np.nonzero(mask.any(axis=1))[0]
        gsel = np.where(mask[tok, 0], g[tok, 0], g[tok, 1]).astype(np.float32)
        idx_lists.append(tok.astype(np.int64))
        gate_lists.append(gsel)

    counts = [len(ix) for ix in idx_lists]

    # Overflow chunking: smallest OV (multiple of 16, <= 64 for the packed
    # PSUM bank) such that single-expert chunks of <= OV tokens fit one per
    # core.  chunks[c] = (expert, slice into idx_lists[expert]).
    chunks = None
    OV = 0
    for ov_try in range(16, 65, 16):
        trial = []
        ok = True
        for e in range(E):
            over = counts[e] - CAP
            pos = CAP
            while over > 0:
                take = min(over, ov_try)
                trial.append((e, pos, pos + take))
                pos += take
                over -= take
        if len(trial) <= NCORES:
            chunks, OV = trial, ov_try
            break
    assert chunks is not None, f"counts {counts} need OV > 64"
    chunks += [None] * (NCORES - len(chunks))
    CTOT = CAP + OV

    if OV not in _compiled:
        _compiled[OV] = _build(OV)
    nc = _compiled[OV]

    bf = ml_dtypes.bfloat16
    xTf = np.ascontiguousarray(x.T).astype(bf)   # [H, T] bf16
    Wgb = [Wg[e].astype(bf) for e in range(E)]
    Wub = [Wu[e].astype(bf) for e in range(E)]
    Wdb = [Wd[e].astype(bf) for e in range(E)]
    zg = np.zeros((H, I), dtype=bf)
    zd = np.zeros((I, H), dtype=bf)
    in_maps = []
    for e in range(E):
        n1 = min(counts[e], CAP)
        xTe = np.zeros((H, CTOT), dtype=bf)
        xTe[:, :n1] = xTf[:, idx_lists[e][:n1]]
        gme = np.zeros((PB, CTOT), dtype=np.float32)
        gme[:, :n1] = gate_lists[e][:n1][None, :]
        gm2e = np.zeros((PB, HB * OV), dtype=np.float32)
        ch = chunks[e]
        if ch is not None:
            e2, a, b = ch
            xTe[:, CAP:CAP + (b - a)] = xTf[:, idx_lists[e2][a:b]]
            g2 = np.zeros((OV,), dtype=np.float32)
            g2[:b - a] = gate_lists[e2][a:b]
            gm2e[:, :] = np.tile(g2, HB)[None, :]
        in_maps.append({
            "xT": xTe,
            "gm": gme,
            "gm2": gm2e,
            "Wg": Wgb[e], "Wu": Wub[e], "Wd": Wdb[e],
            "Wg2": Wgb[ch[0]] if ch else zg,
            "Wu2": Wub[ch[0]] if ch else zg,
            "Wd2": Wdb[ch[0]] if ch else zd,
        })

    trace = bool(int(os.environ.get("MOE_TRACE", "0")))
    trace_cores = (list(range(NCORES))
                   if os.environ.get("MOE_TRACE_ALL") else None)
    last_results = run_bass_kernel_spmd(
        nc, in_maps, core_ids=list(range(NCORES)), trace=trace,
        trace_cores=trace_cores)

    out = np.zeros((T, H), dtype=np.float32)
    for e in range(E):
        yTe = last_results.results[e]["yT"]
        n1 = min(counts[e], CAP)
        out[idx_lists[e][:n1]] += yTe[:, :n1].T
        ch = chunks[e]
        if ch is not None:
            e2, a, b = ch
            out[idx_lists[e2][a:b]] += yTe[:, CAP:CAP + (b - a)].T
    return out
